# revision 1
# baseline (speedup 1.0000x reference)
"""T5-style causal multi-head attention (B=4, S=2048, E=1024, H=16, D=64)
on 8 NeuronCores. Sharding: core c handles batch c//2 and head half c%2
(8 heads). Host sums the two row-parallel partial output projections per
batch.

Device program (SPMD, identical on all cores; data differs):
  stage 1: PE-transpose x blocks -> x^T chunks; fp32r projections
           qT,kT [hd,tok] and v [tok,hd] (bf16, ones column appended).
  stage 2: per head-pair: preload 8*bias+mask tile into PSUM via
           identity-matmul, row-tiled K=64 QK pair accumulates on top,
           ACT exp(x/8) -> bf16 P^T tiles, PV matmul with M=65 fuses the
           softmax denominator; normalize O^T with exp(-ln(den)) via a
           DRAM-broadcast of the reciprocal row.
  stage 3: fp32r output projection partial = O^T.T @ Wo_half.
"""
import sys

sys.path.insert(0, "/opt/trn_rl_repo")

import numpy as np
import ml_dtypes

import concourse.bass as bass
import concourse.mybir as mybir
import concourse.tile as tile
from concourse import bacc
from concourse.bass_utils import run_bass_kernel_spmd
from concourse.masks import make_identity

F32, F32R, BF16 = mybir.dt.float32, mybir.dt.float32r, mybir.dt.bfloat16
AF = mybir.ActivationFunctionType

B, S, E, H, D = 4, 2048, 1024, 16, 64
HL = H // 2          # heads per core
HD = HL * D          # 512, per-core head dims
NUM_BUCKETS, MAX_DISTANCE = 32, 128
NEG = np.float32(-3.0e38)
NT = S // 128        # 16 token blocks
NE = E // 128        # 8 embed chunks

_NC_CACHE = {}


# ---------------------------------------------------------------- host side

def _np_bucket(distance):
    """Mirror reference._relative_position_bucket for causal (distance>=0),
    float32 arithmetic like jnp."""
    max_exact = NUM_BUCKETS // 2  # 16
    is_small = distance < max_exact
    safe = np.maximum(distance, 1).astype(np.float32)
    log_scale = np.log(safe / np.float32(max_exact)).astype(np.float32) / np.float32(
        np.log(np.float32(MAX_DISTANCE / max_exact))
    )
    large = max_exact + (log_scale * np.float32(NUM_BUCKETS - max_exact)).astype(
        np.int32
    )
    large = np.minimum(large, NUM_BUCKETS - 1)
    return np.where(is_small, distance, large)


def _build_btab(rel_bias_half):
    """rel_bias_half [8, 32] -> btab [4 hp, 128 k, 16 m, 2 h, 512 q] bf16
    with 8*bias + causal mask (-3e38). m-index = (4*qc - kb) + 3."""
    qq = np.arange(512)[None, :]
    kk = np.arange(128)[:, None]
    tiles = []
    for mi in range(16):
        m = mi - 3
        dd = 128 * m + qq - kk  # q - k distance, [128, 512]
        bucket = _np_bucket(np.maximum(dd, 0))
        vals = 8.0 * rel_bias_half[:, bucket]  # [8, 128, 512]
        vals = np.where(dd[None] >= 0, vals, NEG)
        tiles.append(vals.astype(np.float32))
    t = np.stack(tiles, axis=0)  # [16 m, 8 h, 128 k, 512 q]
    # -> [hp, k, m, h-in-pair, q]
    t = t.reshape(16, 4, 2, 128, 512).transpose(1, 3, 0, 2, 4)
    return np.ascontiguousarray(t).astype(ml_dtypes.bfloat16)


# -------------------------------------------------------------- device side

def _build_nc(stages=(1, 2, 3)):
    nc = bacc.Bacc(None, target_bir_lowering=False)
    xq_d = nc.dram_tensor("xq", [S, E], F32, kind="ExternalInput")
    xkv_d = nc.dram_tensor("xkv", [S, E], F32, kind="ExternalInput")
    wq_d = nc.dram_tensor("wq", [E, HD], F32, kind="ExternalInput")
    wk_d = nc.dram_tensor("wk", [E, HD], F32, kind="ExternalInput")
    wv_d = nc.dram_tensor("wv", [E, HD], F32, kind="ExternalInput")
    wo_d = nc.dram_tensor("wo", [HD, E], F32, kind="ExternalInput")
    btab_d = nc.dram_tensor("btab", [4, 128, 16, 2, 512], BF16,
                            kind="ExternalInput")
    out_d = nc.dram_tensor("out", [S, E], F32, kind="ExternalOutput")
    rec_d = nc.dram_tensor("rec_scratch", [2, 128, 4, 512], F32)

    with tile.TileContext(nc) as tc:
        with (
            tc.tile_pool(name="const", bufs=1) as pconst,
            tc.tile_pool(name="persist", bufs=1) as pper,
        ):
            ident = pconst.tile([128, 128], F32)
            make_identity(nc, ident)
            identb = pconst.tile([128, 128], BF16)
            nc.vector.tensor_copy(identb, ident)

            qT = pper.tile([128, 4, S], F32R)         # [pair-dims, hp, tok]
            kT = pper.tile([128, 4, S], F32R)
            vA = pper.tile([128, NT, HL * 65], BF16)  # v + ones col per head
            # denominators: head h -> tile h//4, partition 32*(h%4)
            den0 = pper.tile([128, 4, 512], F32)
            den1 = pper.tile([128, 4, 512], F32)

            vAr = vA.rearrange("p t (h c) -> p t h c", c=65)
            nc.vector.memset(vAr[:, :, :, 64:65], 1.0)

            # ---------------- stage 1: transposes + projections
            def transpose_pass(x_dram, prow, pxt, pps, tag):
                """Yields (tp, xT tile [128, NE, 2, 128] f32r) per token pair."""
                for tp in range(NT // 2):
                    row = prow.tile([128, 2, E], F32, tag=f"row{tag}")
                    for j in range(2):
                        t = tp * 2 + j
                        nc.sync.dma_start(out=row[:, j, :],
                                          in_=x_dram[t * 128:(t + 1) * 128, :])
                    xT = pxt.tile([128, NE, 2, 128], F32R, tag=f"xT{tag}")
                    for j in range(2):
                        for e in range(NE):
                            pt = pps.tile([128, 128], F32, tag=f"pt{tag}")
                            nc.tensor.transpose(
                                pt, row[:, j, e * 128:(e + 1) * 128], ident)
                            nc.vector.tensor_copy(xT[:, e, j, :], pt)
                    yield tp, xT

            # pass A: q projection
            if 1 in stages:
             with (
                tc.tile_pool(name="s1wa", bufs=1) as p1w,
                tc.tile_pool(name="s1row", bufs=2) as p1r,
                tc.tile_pool(name="s1xt", bufs=2) as p1x,
                tc.tile_pool(name="psT", bufs=4, space="PSUM") as psT,
                tc.tile_pool(name="psP", bufs=4, space="PSUM") as psP,
            ):
                wq_sb = p1w.tile([128, NE, HD], F32R)
                nc.sync.dma_start(
                    out=wq_sb,
                    in_=wq_d[:].bitcast(F32R).rearrange("(e p) n -> p e n", p=128))
                for tp, xT in transpose_pass(xq_d, p1r, p1x, psT, "q"):
                    for hc in range(4):
                        qps = psP.tile([128, 256], F32, tag="pj")
                        for e in range(NE):
                            nc.tensor.matmul(
                                qps, wq_sb[:, e, hc * 128:(hc + 1) * 128],
                                xT[:, e, :, :], start=(e == 0), stop=(e == NE - 1))
                        nc.vector.tensor_copy(
                            qT[:, hc, tp * 256:(tp + 1) * 256], qps)

            # pass B: k and v projections (shared xkv transpose)
            if 1 in stages:
             with (
                tc.tile_pool(name="s1wb", bufs=1) as p1w,
                tc.tile_pool(name="s1row", bufs=2) as p1r,
                tc.tile_pool(name="s1xt", bufs=2) as p1x,
                tc.tile_pool(name="psT", bufs=2, space="PSUM") as psT,
                tc.tile_pool(name="psP", bufs=4, space="PSUM") as psP,
            ):
                wk_sb = p1w.tile([128, NE, HD], F32R)
                wv_sb = p1w.tile([128, NE, HD], F32R)
                for w_sb, w_dr in ((wk_sb, wk_d), (wv_sb, wv_d)):
                    nc.sync.dma_start(
                        out=w_sb,
                        in_=w_dr[:].bitcast(F32R).rearrange("(e p) n -> p e n", p=128))
                for tp, xT in transpose_pass(xkv_d, p1r, p1x, psT, "k"):
                    for hc in range(4):
                        kps = psP.tile([128, 256], F32, tag="pj")
                        for e in range(NE):
                            nc.tensor.matmul(
                                kps, wk_sb[:, e, hc * 128:(hc + 1) * 128],
                                xT[:, e, :, :], start=(e == 0), stop=(e == NE - 1))
                        nc.vector.tensor_copy(
                            kT[:, hc, tp * 256:(tp + 1) * 256], kps)
                    for j in range(2):
                        vps = psP.tile([128, HD], F32, tag="pj")
                        for e in range(NE):
                            nc.tensor.matmul(
                                vps, xT[:, e, j, :], wv_sb[:, e, :],
                                start=(e == 0), stop=(e == NE - 1))
                        nc.vector.tensor_copy(
                            vAr[:, tp * 2 + j, :, 0:64],
                            vps.rearrange("p (h c) -> p h c", c=64))

            # ---------------- stages 2+3 share the O^T pool
            with tc.tile_pool(name="persist2", bufs=1) as pper2:
                oT = pper2.tile([128, 4, S], F32R)

                if 2 in stages:
                 with (
                    tc.tile_pool(name="s2b", bufs=1) as p2b,
                    tc.tile_pool(name="s2p", bufs=4) as p2p,
                    tc.tile_pool(name="s2r", bufs=4) as p2r,
                    tc.tile_pool(name="psS", bufs=4, space="PSUM") as psS,
                    tc.tile_pool(name="psO", bufs=4, space="PSUM") as psO,
                ):
                    for hp in range(4):
                        b_sb = p2b.tile([128, 16, 2, 512], BF16, tag="btab")
                        nc.sync.dma_start(out=b_sb, in_=btab_d[hp])
                        for qc in range(4):
                            o0 = psO.tile([65, 512], F32, tag="o")
                            o1 = psO.tile([65, 512], F32, tag="o")
                            nkb = 4 * qc + 4
                            for kb in range(nkb):
                                mi = 4 * qc - kb + 3
                                s0 = psS.tile([128, 512], F32, tag="s")
                                s1 = psS.tile([128, 512], F32, tag="s")
                                nc.tensor.matmul(s0, identb, b_sb[:, mi, 0, :],
                                                 start=True, stop=False)
                                nc.tensor.matmul(s1, identb, b_sb[:, mi, 1, :],
                                                 start=True, stop=False)
                                nc.tensor.matmul(
                                    s0, kT[0:64, hp, kb * 128:(kb + 1) * 128],
                                    qT[0:64, hp, qc * 512:(qc + 1) * 512],
                                    start=False, stop=True)
                                nc.tensor.matmul(
                                    s1, kT[64:128, hp, kb * 128:(kb + 1) * 128],
                                    qT[64:128, hp, qc * 512:(qc + 1) * 512],
                                    start=False, stop=True)
                                p0 = p2p.tile([128, 512], BF16, tag="p")
                                p1 = p2p.tile([128, 512], BF16, tag="p")
                                nc.scalar.activation(p0, s0, AF.Exp, scale=0.125)
                                nc.scalar.activation(p1, s1, AF.Exp, scale=0.125)
                                h0, h1 = 2 * hp, 2 * hp + 1
                                nc.tensor.matmul(
                                    o0, vA[:, kb, h0 * 65:(h0 + 1) * 65], p0,
                                    start=(kb == 0), stop=(kb == nkb - 1))
                                nc.tensor.matmul(
                                    o1, vA[:, kb, h1 * 65:(h1 + 1) * 65], p1,
                                    start=(kb == 0), stop=(kb == nkb - 1))
                            # epilogue: stash raw O^T + denominators
                            for hh, ops_o in ((0, o0), (1, o1)):
                                h = 2 * hp + hh
                                dt_, pr = (den0, den1)[h // 4], 32 * (h % 4)
                                nc.vector.tensor_copy(
                                    dt_[pr:pr + 1, qc, :], ops_o[64:65, :])
                                nc.vector.tensor_copy(
                                    oT[hh * 64:(hh + 1) * 64, hp,
                                       qc * 512:(qc + 1) * 512], ops_o[0:64, :])

                    # reciprocals: rec = exp(-ln(den)), then DRAM broadcast
                    for ti, dt_ in enumerate((den0, den1)):
                        nc.scalar.activation(dt_, dt_, AF.Ln)
                        nc.scalar.activation(dt_, dt_, AF.Exp, scale=-1.0)
                        nc.sync.dma_start(out=rec_d[ti], in_=dt_)
                    for hp in range(4):
                        for hh in range(2):
                            h = 2 * hp + hh
                            for qc in range(4):
                                rep = p2r.tile([128, 512], F32R, tag="rep")
                                src = rec_d[h // 4, 32 * (h % 4), qc, :]
                                nc.sync.dma_start(
                                    out=rep,
                                    in_=bass.AP(
                                        tensor=src.tensor, offset=src.offset,
                                        ap=[[0, 128]] + src.ap,
                                    ).bitcast(F32R))
                                sl = oT[hh * 64:(hh + 1) * 64, hp,
                                        qc * 512:(qc + 1) * 512]
                                nc.vector.tensor_tensor(
                                    out=sl, in0=sl,
                                    in1=rep[hh * 64:(hh + 1) * 64, :],
                                    op=mybir.AluOpType.mult)

                # ---------------- stage 3: output projection
                if 3 in stages:
                 with (
                    tc.tile_pool(name="s3w", bufs=1) as p3w,
                    tc.tile_pool(name="s3o", bufs=3) as p3o,
                    tc.tile_pool(name="psF", bufs=4, space="PSUM") as psF,
                ):
                    wo_sb = p3w.tile([128, 4, E], F32R)
                    nc.sync.dma_start(
                        out=wo_sb,
                        in_=wo_d[:].bitcast(F32R).rearrange(
                            "(g p) n -> p g n", p=128))
                    for t in range(NT):
                        oev = p3o.tile([128, E], F32, tag="oev")
                        for ec in range(2):
                            ops = psF.tile([128, 512], F32, tag="ops")
                            for hp in range(4):
                                nc.tensor.matmul(
                                    ops, oT[:, hp, t * 128:(t + 1) * 128],
                                    wo_sb[:, hp, ec * 512:(ec + 1) * 512],
                                    start=(hp == 0), stop=(hp == 3))
                            nc.vector.tensor_copy(
                                oev[:, ec * 512:(ec + 1) * 512], ops)
                        nc.sync.dma_start(
                            out=out_d[t * 128:(t + 1) * 128, :], in_=oev)

    nc.compile()
    return nc


def _get_nc():
    if "nc" not in _NC_CACHE:
        _NC_CACHE["nc"] = _build_nc()
    return _NC_CACHE["nc"]


def kernel(inputs_q, inputs_kv, mask, Wq, Wk, Wv, Wo, rel_bias):
    inputs_q = np.asarray(inputs_q, dtype=np.float32)
    inputs_kv = np.asarray(inputs_kv, dtype=np.float32)
    Wq = np.asarray(Wq, dtype=np.float32)
    Wk = np.asarray(Wk, dtype=np.float32)
    Wv = np.asarray(Wv, dtype=np.float32)
    Wo = np.asarray(Wo, dtype=np.float32)
    rel_bias = np.asarray(rel_bias, dtype=np.float32)

    nc = _get_nc()
    btabs = [_build_btab(rel_bias[0:HL]), _build_btab(rel_bias[HL:])]
    in_maps = []
    for c in range(8):
        b, half = c // 2, c % 2
        sl = slice(half * HD, (half + 1) * HD)
        in_maps.append({
            "xq": np.ascontiguousarray(inputs_q[b]),
            "xkv": np.ascontiguousarray(inputs_kv[b]),
            "wq": np.ascontiguousarray(Wq[:, sl]),
            "wk": np.ascontiguousarray(Wk[:, sl]),
            "wv": np.ascontiguousarray(Wv[:, sl]),
            "wo": np.ascontiguousarray(Wo[sl, :]),
            "btab": btabs[half],
        })
    res = run_bass_kernel_spmd(nc, in_maps, core_ids=list(range(8)))
    out = np.stack(
        [res.results[2 * b]["out"] + res.results[2 * b + 1]["out"]
         for b in range(B)])
    return out.astype(np.float32)



# revision 5
# speedup vs baseline: 1.2894x; 1.2894x over previous
"""T5-style causal multi-head attention (B=4, S=2048, E=1024, H=16, D=64)
on 8 NeuronCores. Sharding: core c handles batch c//2 and head half c%2
(8 heads). Host sums the two row-parallel partial output projections per
batch.

v2 design notes:
- The T5 bias saturates at bucket 31 for distance >= 113; that far-field
  value is constant per head across all keys of a query row, so it cancels
  in softmax. Far blocks therefore need NO bias at all; near blocks
  (mi <= 4) add a small fp8 table of 8*(bias[bucket]-bias[31]) (+ mask)
  via an fp8 identity-matmul PSUM preload. The near table for all 4 head
  pairs is SBUF-resident (no per-hp DMA bubbles).
- QK scores for both heads accumulate into one 2-bank PSUM tile
  [128,1024]; a single ACT instruction does exp over both heads.
- Softmax denominators come free via a ones-column in the PV lhsT (m=65);
  normalization happens per (hp,qc) tile: DVE reciprocal of the den row,
  DRAM-broadcast of the reciprocal, and a fused multiply during the PSUM
  drain of O^T.
- Stage 1: PE transposes x (f32r, 1.5 cyc/row); Q path in bf16, K/V path
  in f32r. PSUM->SBUF copies are split between ACT and DVE engines.
"""
import sys

sys.path.insert(0, "/opt/trn_rl_repo")

import numpy as np
import ml_dtypes

import concourse.bass as bass
import concourse.mybir as mybir
import concourse.tile as tile
from concourse import bacc
from concourse.bass_utils import run_bass_kernel_spmd
from concourse.masks import make_identity

F32, F32R, BF16 = mybir.dt.float32, mybir.dt.float32r, mybir.dt.bfloat16
F8 = mybir.dt.float8e4
AF = mybir.ActivationFunctionType

B, S, E, H, D = 4, 2048, 1024, 16, 64
HL = H // 2          # heads per core
HD = HL * D          # 512, per-core head dims
NUM_BUCKETS, MAX_DISTANCE = 32, 128
NEG8 = np.float32(-240.0)   # min-ish of fp8 e4m3 (IEEE): kills exp after /8
NT = S // 128        # 16 token blocks
NE = E // 8 // 16    # placeholder; real NE below
NE = E // 128        # 8 embed chunks
NQ = 4               # token quads (512 tokens each)

_NC_CACHE = {}


# ---------------------------------------------------------------- host side

def _np_bucket(distance):
    """Mirror reference._relative_position_bucket for causal (distance>=0),
    float32 arithmetic like jnp."""
    max_exact = NUM_BUCKETS // 2  # 16
    is_small = distance < max_exact
    safe = np.maximum(distance, 1).astype(np.float32)
    log_scale = np.log(safe / np.float32(max_exact)).astype(np.float32) / np.float32(
        np.log(np.float32(MAX_DISTANCE / max_exact))
    )
    large = max_exact + (log_scale * np.float32(NUM_BUCKETS - max_exact)).astype(
        np.int32
    )
    large = np.minimum(large, NUM_BUCKETS - 1)
    return np.where(is_small, distance, large)


def _build_btab_near(rel_bias_half):
    """rel_bias_half [8, 32] -> near-table [128 k, 4 hp, 5 mi, 2 h, 512 q]
    fp8, holding 8*(bias[bucket] - bias[31]) for valid, -240 for masked.
    The -bias[31] shift is the constant far-field bias, which cancels in
    softmax. m-index mi = (4*qc - kb) + 3; only mi <= 4 blocks need it."""
    rb = np.asarray(rel_bias_half, dtype=np.float32)        # [8, 32]
    qq = np.arange(512)[None, :]
    kk = np.arange(128)[:, None]
    tiles = []
    for mi in range(5):
        m = mi - 3
        dd = 128 * m + qq - kk                              # [128, 512]
        bucket = _np_bucket(np.maximum(dd, 0))
        vals = 8.0 * (rb[:, bucket] - rb[:, 31][:, None, None])   # [8,128,512]
        vals = np.where(dd[None] >= 0, vals, NEG8)
        tiles.append(vals.astype(np.float32))
    t = np.stack(tiles, axis=0)                             # [5, 8h, 128, 512]
    t = t.reshape(5, 4, 2, 128, 512).transpose(3, 1, 0, 2, 4)  # [128,4,5,2,512]
    return np.ascontiguousarray(t).astype(ml_dtypes.float8_e4m3)


def make_in_maps(inputs_q, inputs_kv, Wq, Wk, Wv, Wo, rel_bias):
    inputs_q = np.asarray(inputs_q, dtype=np.float32)
    inputs_kv = np.asarray(inputs_kv, dtype=np.float32)
    Wq = np.asarray(Wq, dtype=np.float32)
    Wk = np.asarray(Wk, dtype=np.float32)
    Wv = np.asarray(Wv, dtype=np.float32)
    Wo = np.asarray(Wo, dtype=np.float32)
    rel_bias = np.asarray(rel_bias, dtype=np.float32)
    btabs = [_build_btab_near(rel_bias[0:HL]), _build_btab_near(rel_bias[HL:])]
    in_maps = []
    for c in range(8):
        b, half = c // 2, c % 2
        sl = slice(half * HD, (half + 1) * HD)
        in_maps.append({
            "xq": np.ascontiguousarray(inputs_q[b]),
            "xkv": np.ascontiguousarray(inputs_kv[b]),
            "wq": np.ascontiguousarray(Wq[:, sl]).astype(ml_dtypes.bfloat16),
            "wk": np.ascontiguousarray(Wk[:, sl]),
            "wv": np.ascontiguousarray(Wv[:, sl]),
            "wo": np.ascontiguousarray(Wo[sl, :]),
            "btab": btabs[half],
        })
    return in_maps


# -------------------------------------------------------------- device side

def _build_nc():
    nc = bacc.Bacc(None, target_bir_lowering=False)
    xq_d = nc.dram_tensor("xq", [S, E], F32, kind="ExternalInput")
    xkv_d = nc.dram_tensor("xkv", [S, E], F32, kind="ExternalInput")
    wq_d = nc.dram_tensor("wq", [E, HD], BF16, kind="ExternalInput")
    wk_d = nc.dram_tensor("wk", [E, HD], F32, kind="ExternalInput")
    wv_d = nc.dram_tensor("wv", [E, HD], F32, kind="ExternalInput")
    wo_d = nc.dram_tensor("wo", [HD, E], F32, kind="ExternalInput")
    btab_d = nc.dram_tensor("btab", [128, 4, 5, 2, 512], F8,
                            kind="ExternalInput")
    out_d = nc.dram_tensor("out", [S, E], F32, kind="ExternalOutput")
    rec_d = nc.dram_tensor("rec_scratch", [4, 4, 2, 512], F32)

    with tile.TileContext(nc) as tc:
        with (
            tc.tile_pool(name="const", bufs=1) as pconst,
            tc.tile_pool(name="persist", bufs=1) as pper,
        ):
            ident = pconst.tile([128, 128], F32)
            make_identity(nc, ident)
            identf8 = pconst.tile([128, 128], F8)
            nc.vector.tensor_copy(identf8, ident)
            identr = pconst.tile([128, 128], F32R)
            nc.vector.tensor_copy(identr, ident)

            qT = pper.tile([128, 4, S], BF16)         # [pair-dims, hp, tok]
            kT = pper.tile([128, 4, S], BF16)
            vA = pper.tile([128, NT, HL * 65], BF16)  # v + ones col per head

            vAr = vA.rearrange("p t (h c) -> p t h c", c=65)
            nc.vector.memset(vAr[:, :, :, 64:65], 1.0)

            # ---------------- stage 1: transposes + projections
            def transpose_quads(x_dram, prow, pxt, pps, xt_dtype, tag):
                """Yields (tq, xT tile [128, NE, 4, 128]) per token quad."""
                for tq in range(NQ):
                    rows = []
                    for rj in range(2):
                        r = prow.tile([128, 2, E], F32R, tag=f"row{tag}")
                        for j in range(2):
                            t = tq * 4 + rj * 2 + j
                            nc.sync.dma_start(
                                out=r[:, j, :],
                                in_=x_dram[t * 128:(t + 1) * 128, :]
                                .bitcast(F32R))
                        rows.append(r)
                    xT = pxt.tile([128, NE, 4, 128], xt_dtype, tag=f"xT{tag}")
                    for e in range(NE):
                        pt = pps.tile([128, 512], F32R, tag=f"pt{tag}")
                        for rj in range(2):
                            for j in range(2):
                                nc.tensor.transpose(
                                    pt[:, (rj * 2 + j) * 128:(rj * 2 + j + 1) * 128],
                                    rows[rj][:, j, e * 128:(e + 1) * 128],
                                    identr)
                        # PSUM->SBUF copy on ACT (idle during stage 1)
                        nc.scalar.copy(xT[:, e], pt.bitcast(F32))
                    yield tq, xT

            # pass A: q projection (bf16 path)
            with (
                tc.tile_pool(name="s1wa", bufs=1) as p1wa,
                tc.tile_pool(name="s1wb", bufs=1) as p1wb,
            ):
                wq_sb = p1wa.tile([128, NE, HD], BF16)
                nc.sync.dma_start(
                    out=wq_sb,
                    in_=wq_d[:].rearrange("(e p) n -> p e n", p=128))
                wk_sb = p1wb.tile([128, NE, HD], F32R)
                wv_sb = p1wb.tile([128, NE, HD], F32R)
                for w_sb, w_dr in ((wk_sb, wk_d), (wv_sb, wv_d)):
                    nc.sync.dma_start(
                        out=w_sb,
                        in_=w_dr[:].bitcast(F32R).rearrange(
                            "(e p) n -> p e n", p=128))

                with (
                    tc.tile_pool(name="s1row", bufs=3) as p1r,
                    tc.tile_pool(name="s1xt", bufs=2) as p1x,
                    tc.tile_pool(name="psT", bufs=3, space="PSUM") as psT,
                    tc.tile_pool(name="psP", bufs=3, space="PSUM") as psP,
                ):
                    for tq, xT in transpose_quads(xq_d, p1r, p1x, psT, BF16, "q"):
                        for hc in range(4):
                            qps = psP.tile([128, 512], F32, tag="pj")
                            for e in range(NE):
                                nc.tensor.matmul(
                                    qps, wq_sb[:, e, hc * 128:(hc + 1) * 128],
                                    xT[:, e], start=(e == 0), stop=(e == NE - 1))
                            nc.vector.tensor_copy(
                                qT[:, hc, tq * 512:(tq + 1) * 512], qps)

                # pass B: k and v projections (shared xkv transpose, f32r)
                with (
                    tc.tile_pool(name="s1rowb", bufs=3) as p1r,
                    tc.tile_pool(name="s1xtb", bufs=2) as p1x,
                    tc.tile_pool(name="psTb", bufs=3, space="PSUM") as psT,
                    tc.tile_pool(name="psPb", bufs=4, space="PSUM") as psP,
                ):
                    for tq, xT in transpose_quads(xkv_d, p1r, p1x, psT, F32R, "k"):
                        xTr = xT  # f32r tile
                        for hc in range(4):
                            kps = psP.tile([128, 512], F32, tag="pj")
                            for e in range(NE):
                                nc.tensor.matmul(
                                    kps, wk_sb[:, e, hc * 128:(hc + 1) * 128],
                                    xTr[:, e], start=(e == 0), stop=(e == NE - 1))
                            nc.vector.tensor_copy(
                                kT[:, hc, tq * 512:(tq + 1) * 512], kps)
                        for j in range(4):
                            t = tq * 4 + j
                            vps = psP.tile([128, HD], F32, tag="pj")
                            for e in range(NE):
                                nc.tensor.matmul(
                                    vps, xTr[:, e, j, :], wv_sb[:, e, :],
                                    start=(e == 0), stop=(e == NE - 1))
                            nc.vector.tensor_copy(
                                vAr[:, t, :, 0:64],
                                vps.rearrange("p (h c) -> p h c", c=64))

            # ---------------- stages 2+3
            with tc.tile_pool(name="s2per", bufs=1) as p2per:
                oT = p2per.tile([128, 4, S], F32R)
                b_sb = p2per.tile([128, 4, 5, 2, 512], F8)
                nc.sync.dma_start(out=b_sb, in_=btab_d[:])
                wo_sb = p2per.tile([128, 4, E], F32R)
                nc.sync.dma_start(
                    out=wo_sb,
                    in_=wo_d[:].bitcast(F32R).rearrange(
                        "(g p) n -> p g n", p=128))

                with (
                    tc.tile_pool(name="s2p", bufs=3) as p2p,
                    tc.tile_pool(name="s2rec", bufs=2) as p2rc,
                    tc.tile_pool(name="s2rep", bufs=2) as p2rp,
                    tc.tile_pool(name="psS", bufs=2, space="PSUM") as psS,
                    tc.tile_pool(name="psO", bufs=4, space="PSUM") as psO,
                ):
                 for hp in range(4):
                    for qc in range(4):
                        o0 = psO.tile([65, 512], F32, tag="o")
                        o1 = psO.tile([65, 512], F32, tag="o")
                        nkb = 4 * qc + 4
                        for kb in range(nkb):
                            mi = 4 * qc - kb + 3
                            s2 = psS.tile([128, 1024], F32, tag="s")
                            near = mi <= 4
                            # diagonal-straddling blocks (mi<=3) only touch
                            # queries q >= w0; skip the fully-masked columns
                            w0 = 128 * (3 - mi) if mi <= 3 else 0
                            if near:
                                for hh in range(2):
                                    nc.tensor.matmul(
                                        s2[:, hh * 512 + w0:hh * 512 + 512],
                                        identf8,
                                        b_sb[:, hp, mi, hh, w0:512],
                                        start=True, stop=False)
                            for hh in range(2):
                                nc.tensor.matmul(
                                    s2[:, hh * 512 + w0:hh * 512 + 512],
                                    kT[hh * 64:hh * 64 + 64, hp,
                                       kb * 128:(kb + 1) * 128],
                                    qT[hh * 64:hh * 64 + 64, hp,
                                       qc * 512 + w0:(qc + 1) * 512],
                                    start=not near, stop=True)
                            p4 = p2p.tile([128, 1024], BF16, tag="p")
                            s2v = s2.rearrange("p (h n) -> p h n", n=512)
                            p4v = p4.rearrange("p (h n) -> p h n", n=512)
                            nc.scalar.activation(p4v[:, :, w0:512],
                                                 s2v[:, :, w0:512],
                                                 AF.Exp, scale=0.125)
                            h0, h1 = 2 * hp, 2 * hp + 1
                            nc.tensor.matmul(
                                o0[:, w0:512],
                                vA[:, kb, h0 * 65:(h0 + 1) * 65],
                                p4[:, w0:512],
                                start=(kb == 0), stop=(kb == nkb - 1),
                                skip_group_check=(w0 > 0))
                            nc.tensor.matmul(
                                o1[:, w0:512],
                                vA[:, kb, h1 * 65:(h1 + 1) * 65],
                                p4[:, 512 + w0:1024],
                                start=(kb == 0), stop=(kb == nkb - 1),
                                skip_group_check=(w0 > 0))
                        # epilogue: normalize + drain O^T
                        for hh, ops_o in ((0, o0), (1, o1)):
                            rec = p2rc.tile([65, 512], F32, tag="rec")
                            nc.vector.reciprocal(rec[64:65, :], ops_o[64:65, :])
                            nc.sync.dma_start(out=rec_d[hp, qc, hh],
                                              in_=rec[64:65, :])
                            rep = p2rp.tile([64, 512], F32, tag="rep")
                            src = rec_d[hp, qc, hh, :]
                            nc.sync.dma_start(
                                out=rep,
                                in_=bass.AP(
                                    tensor=src.tensor, offset=src.offset,
                                    ap=[[0, 64]] + src.ap,
                                ))
                            nc.vector.tensor_tensor(
                                out=oT[hh * 64:(hh + 1) * 64, hp,
                                       qc * 512:(qc + 1) * 512],
                                in0=ops_o[0:64, :], in1=rep,
                                op=mybir.AluOpType.mult)

                # ---------------- stage 3: output projection
                with (
                    tc.tile_pool(name="s3o", bufs=3) as p3o,
                    tc.tile_pool(name="psF", bufs=4, space="PSUM") as psF,
                ):
                    for t in range(NT):
                        oev = p3o.tile([128, E], F32, tag="oev")
                        for ec in range(2):
                            ops = psF.tile([128, 512], F32, tag="ops")
                            for hp in range(4):
                                nc.tensor.matmul(
                                    ops, oT[:, hp, t * 128:(t + 1) * 128],
                                    wo_sb[:, hp, ec * 512:(ec + 1) * 512],
                                    start=(hp == 0), stop=(hp == 3))
                            nc.scalar.copy(
                                oev[:, ec * 512:(ec + 1) * 512], ops)
                        nc.sync.dma_start(
                            out=out_d[t * 128:(t + 1) * 128, :], in_=oev)

    nc.compile()
    return nc


def _get_nc():
    if "nc" not in _NC_CACHE:
        _NC_CACHE["nc"] = _build_nc()
    return _NC_CACHE["nc"]


def kernel(inputs_q, inputs_kv, mask, Wq, Wk, Wv, Wo, rel_bias):
    nc = _get_nc()
    in_maps = make_in_maps(inputs_q, inputs_kv, Wq, Wk, Wv, Wo, rel_bias)
    res = run_bass_kernel_spmd(nc, in_maps, core_ids=list(range(8)))
    out = np.stack(
        [res.results[2 * b]["out"] + res.results[2 * b + 1]["out"]
         for b in range(B)])
    return out.astype(np.float32)


# revision 6
# speedup vs baseline: 1.4373x; 1.1146x over previous
"""T5-style causal multi-head attention (B=4, S=2048, E=1024, H=16, D=64)
on 8 NeuronCores. Sharding: core c handles batch c//2 and head half c%2
(8 heads). Host sums the two row-parallel partial output projections per
batch.

v2 design notes:
- The T5 bias saturates at bucket 31 for distance >= 113; that far-field
  value is constant per head across all keys of a query row, so it cancels
  in softmax. Far blocks therefore need NO bias at all; near blocks
  (mi <= 4) add a small fp8 table of 8*(bias[bucket]-bias[31]) (+ mask)
  via an fp8 identity-matmul PSUM preload. The near table for all 4 head
  pairs is SBUF-resident (no per-hp DMA bubbles).
- QK scores for both heads accumulate into one 2-bank PSUM tile
  [128,1024]; a single ACT instruction does exp over both heads.
- Softmax denominators come free via a ones-column in the PV lhsT (m=65);
  normalization happens per (hp,qc) tile: DVE reciprocal of the den row,
  DRAM-broadcast of the reciprocal, and a fused multiply during the PSUM
  drain of O^T.
- Stage 1: PE transposes x (f32r, 1.5 cyc/row); Q path in bf16, K/V path
  in f32r. PSUM->SBUF copies are split between ACT and DVE engines.
"""
import sys

sys.path.insert(0, "/opt/trn_rl_repo")

import numpy as np
import ml_dtypes

import concourse.bass as bass
import concourse.mybir as mybir
import concourse.tile as tile
from concourse import bacc
from concourse.bass_utils import run_bass_kernel_spmd
from concourse.masks import make_identity

F32, F32R, BF16 = mybir.dt.float32, mybir.dt.float32r, mybir.dt.bfloat16
F8 = mybir.dt.float8e4
AF = mybir.ActivationFunctionType

B, S, E, H, D = 4, 2048, 1024, 16, 64
HL = H // 2          # heads per core
HD = HL * D          # 512, per-core head dims
NUM_BUCKETS, MAX_DISTANCE = 32, 128
NEG8 = np.float32(-240.0)   # min-ish of fp8 e4m3 (IEEE): kills exp after /8
NT = S // 128        # 16 token blocks
NE = E // 8 // 16    # placeholder; real NE below
NE = E // 128        # 8 embed chunks
NQ = 4               # token quads (512 tokens each)

_NC_CACHE = {}


# ---------------------------------------------------------------- host side

def _np_bucket(distance):
    """Mirror reference._relative_position_bucket for causal (distance>=0),
    float32 arithmetic like jnp."""
    max_exact = NUM_BUCKETS // 2  # 16
    is_small = distance < max_exact
    safe = np.maximum(distance, 1).astype(np.float32)
    log_scale = np.log(safe / np.float32(max_exact)).astype(np.float32) / np.float32(
        np.log(np.float32(MAX_DISTANCE / max_exact))
    )
    large = max_exact + (log_scale * np.float32(NUM_BUCKETS - max_exact)).astype(
        np.int32
    )
    large = np.minimum(large, NUM_BUCKETS - 1)
    return np.where(is_small, distance, large)


def _build_btab_near(rel_bias_half):
    """rel_bias_half [8, 32] -> near-table [128 k, 4 hp, 5 mi, 2 h, 512 q]
    fp8, holding 8*(bias[bucket] - bias[31]) for valid, -240 for masked.
    The -bias[31] shift is the constant far-field bias, which cancels in
    softmax. m-index mi = (4*qc - kb) + 3; only mi <= 4 blocks need it."""
    rb = np.asarray(rel_bias_half, dtype=np.float32)        # [8, 32]
    qq = np.arange(512)[None, :]
    kk = np.arange(128)[:, None]
    tiles = []
    for mi in range(5):
        m = mi - 3
        dd = 128 * m + qq - kk                              # [128, 512]
        bucket = _np_bucket(np.maximum(dd, 0))
        vals = 8.0 * (rb[:, bucket] - rb[:, 31][:, None, None])   # [8,128,512]
        vals = np.where(dd[None] >= 0, vals, NEG8)
        tiles.append(vals.astype(np.float32))
    t = np.stack(tiles, axis=0)                             # [5, 8h, 128, 512]
    t = t.reshape(5, 4, 2, 128, 512).transpose(3, 1, 0, 2, 4)  # [128,4,5,2,512]
    return np.ascontiguousarray(t).astype(ml_dtypes.float8_e4m3)


def make_in_maps(inputs_q, inputs_kv, Wq, Wk, Wv, Wo, rel_bias):
    inputs_q = np.asarray(inputs_q, dtype=np.float32)
    inputs_kv = np.asarray(inputs_kv, dtype=np.float32)
    Wq = np.asarray(Wq, dtype=np.float32)
    Wk = np.asarray(Wk, dtype=np.float32)
    Wv = np.asarray(Wv, dtype=np.float32)
    Wo = np.asarray(Wo, dtype=np.float32)
    rel_bias = np.asarray(rel_bias, dtype=np.float32)
    btabs = [_build_btab_near(rel_bias[0:HL]), _build_btab_near(rel_bias[HL:])]
    in_maps = []
    for c in range(8):
        b, half = c // 2, c % 2
        sl = slice(half * HD, (half + 1) * HD)
        in_maps.append({
            "xq": np.ascontiguousarray(inputs_q[b]),
            "xkv": np.ascontiguousarray(inputs_kv[b]),
            "wq": np.ascontiguousarray(Wq[:, sl]).astype(ml_dtypes.bfloat16),
            "wk": np.ascontiguousarray(Wk[:, sl]),
            "wv": np.ascontiguousarray(Wv[:, sl]),
            "wo": np.ascontiguousarray(Wo[sl, :]),
            "btab": btabs[half],
        })
    return in_maps


# -------------------------------------------------------------- device side

def _build_nc():
    nc = bacc.Bacc(None, target_bir_lowering=False)
    xq_d = nc.dram_tensor("xq", [S, E], F32, kind="ExternalInput")
    xkv_d = nc.dram_tensor("xkv", [S, E], F32, kind="ExternalInput")
    wq_d = nc.dram_tensor("wq", [E, HD], BF16, kind="ExternalInput")
    wk_d = nc.dram_tensor("wk", [E, HD], F32, kind="ExternalInput")
    wv_d = nc.dram_tensor("wv", [E, HD], F32, kind="ExternalInput")
    wo_d = nc.dram_tensor("wo", [HD, E], F32, kind="ExternalInput")
    btab_d = nc.dram_tensor("btab", [128, 4, 5, 2, 512], F8,
                            kind="ExternalInput")
    out_d = nc.dram_tensor("out", [S, E], F32, kind="ExternalOutput")
    rec_d = nc.dram_tensor("rec_scratch", [4, 4, 2, 512], F32)

    with tile.TileContext(nc) as tc:
        with (
            tc.tile_pool(name="const", bufs=1) as pconst,
            tc.tile_pool(name="persist", bufs=1) as pper,
        ):
            ident = pconst.tile([128, 128], F32)
            make_identity(nc, ident)
            identf8 = pconst.tile([128, 128], F8)
            nc.vector.tensor_copy(identf8, ident)
            identr = pconst.tile([128, 128], F32R)
            nc.vector.tensor_copy(identr, ident)

            qT = pper.tile([128, 4, S], BF16)         # [pair-dims, hp, tok]
            kT = pper.tile([128, 4, S], BF16)
            vA = pper.tile([128, NT, HL * 65], BF16)  # v + ones col per head

            vAr = vA.rearrange("p t (h c) -> p t h c", c=65)
            nc.vector.memset(vAr[:, :, :, 64:65], 1.0)

            # ---------------- stage 1: transposes + projections
            def transpose_quads(x_dram, prow, pxt, pps, xt_dtype, tag):
                """Yields (tq, xT tile [128, NE, 4, 128]) per token quad."""
                for tq in range(NQ):
                    rows = []
                    for rj in range(2):
                        r = prow.tile([128, 2, E], F32R, tag=f"row{tag}")
                        for j in range(2):
                            t = tq * 4 + rj * 2 + j
                            nc.sync.dma_start(
                                out=r[:, j, :],
                                in_=x_dram[t * 128:(t + 1) * 128, :]
                                .bitcast(F32R))
                        rows.append(r)
                    xT = pxt.tile([128, NE, 4, 128], xt_dtype, tag=f"xT{tag}")
                    for e in range(NE):
                        pt = pps.tile([128, 512], F32R, tag=f"pt{tag}")
                        for rj in range(2):
                            for j in range(2):
                                nc.tensor.transpose(
                                    pt[:, (rj * 2 + j) * 128:(rj * 2 + j + 1) * 128],
                                    rows[rj][:, j, e * 128:(e + 1) * 128],
                                    identr)
                        # PSUM->SBUF copies split between ACT and DVE
                        if e % 2 == 0:
                            nc.scalar.copy(xT[:, e], pt.bitcast(F32))
                        else:
                            nc.vector.tensor_copy(xT[:, e], pt.bitcast(F32))
                    yield tq, xT

            # pass A: q projection (bf16 path)
            with (
                tc.tile_pool(name="s1wa", bufs=1) as p1wa,
                tc.tile_pool(name="s1wb", bufs=1) as p1wb,
            ):
                wq_sb = p1wa.tile([128, NE, HD], BF16)
                nc.sync.dma_start(
                    out=wq_sb,
                    in_=wq_d[:].rearrange("(e p) n -> p e n", p=128))
                wk_sb = p1wb.tile([128, NE, HD], F32R)
                wv_sb = p1wb.tile([128, NE, HD], F32R)
                for w_sb, w_dr in ((wk_sb, wk_d), (wv_sb, wv_d)):
                    nc.sync.dma_start(
                        out=w_sb,
                        in_=w_dr[:].bitcast(F32R).rearrange(
                            "(e p) n -> p e n", p=128))

                with (
                    tc.tile_pool(name="s1row", bufs=3) as p1r,
                    tc.tile_pool(name="s1xt", bufs=2) as p1x,
                    tc.tile_pool(name="psT", bufs=3, space="PSUM") as psT,
                    tc.tile_pool(name="psP", bufs=3, space="PSUM") as psP,
                ):
                    for tq, xT in transpose_quads(xq_d, p1r, p1x, psT, BF16, "q"):
                        for hc in range(4):
                            qps = psP.tile([128, 512], F32, tag="pj")
                            for e in range(NE):
                                nc.tensor.matmul(
                                    qps, wq_sb[:, e, hc * 128:(hc + 1) * 128],
                                    xT[:, e], start=(e == 0), stop=(e == NE - 1))
                            nc.vector.tensor_copy(
                                qT[:, hc, tq * 512:(tq + 1) * 512], qps)

                # pass B: k and v projections (shared xkv transpose, f32r)
                with (
                    tc.tile_pool(name="s1rowb", bufs=3) as p1r,
                    tc.tile_pool(name="s1xtb", bufs=2) as p1x,
                    tc.tile_pool(name="psTb", bufs=3, space="PSUM") as psT,
                    tc.tile_pool(name="psPb", bufs=4, space="PSUM") as psP,
                ):
                    for tq, xT in transpose_quads(xkv_d, p1r, p1x, psT, F32R, "k"):
                        xTr = xT  # f32r tile
                        for hc in range(4):
                            kps = psP.tile([128, 512], F32, tag="pj")
                            for e in range(NE):
                                nc.tensor.matmul(
                                    kps, wk_sb[:, e, hc * 128:(hc + 1) * 128],
                                    xTr[:, e], start=(e == 0), stop=(e == NE - 1))
                            nc.vector.tensor_copy(
                                kT[:, hc, tq * 512:(tq + 1) * 512], kps)
                        for j in range(4):
                            t = tq * 4 + j
                            vps = psP.tile([128, HD], F32, tag="pj")
                            for e in range(NE):
                                nc.tensor.matmul(
                                    vps, xTr[:, e, j, :], wv_sb[:, e, :],
                                    start=(e == 0), stop=(e == NE - 1))
                            nc.vector.tensor_copy(
                                vAr[:, t, :, 0:64],
                                vps.rearrange("p (h c) -> p h c", c=64))

            # ---------------- stages 2+3
            with tc.tile_pool(name="s2per", bufs=1) as p2per:
                oT = p2per.tile([128, 4, S], F32R)
                b_sb = p2per.tile([128, 4, 5, 2, 512], F8)
                nc.sync.dma_start(out=b_sb, in_=btab_d[:])
                wo_sb = p2per.tile([128, 4, E], F32R)
                nc.sync.dma_start(
                    out=wo_sb,
                    in_=wo_d[:].bitcast(F32R).rearrange(
                        "(g p) n -> p g n", p=128))

                with (
                    tc.tile_pool(name="s2p", bufs=3) as p2p,
                    tc.tile_pool(name="s2rec", bufs=2) as p2rc,
                    tc.tile_pool(name="s2rep", bufs=2) as p2rp,
                    tc.tile_pool(name="psS", bufs=2, space="PSUM") as psS,
                    tc.tile_pool(name="psO", bufs=4, space="PSUM") as psO,
                ):
                 for hp in range(4):
                    for qc in range(4):
                        o0 = psO.tile([65, 512], F32, tag="o")
                        o1 = psO.tile([65, 512], F32, tag="o")
                        nkb = 4 * qc + 4
                        h0, h1 = 2 * hp, 2 * hp + 1

                        def issue_pv(kb, p4, w0, o0=o0, o1=o1, nkb=nkb,
                                     h0=h0, h1=h1):
                            nc.tensor.matmul(
                                o0[:, w0:512],
                                vA[:, kb, h0 * 65:(h0 + 1) * 65],
                                p4[:, w0:512],
                                start=(kb == 0), stop=(kb == nkb - 1),
                                skip_group_check=(w0 > 0))
                            nc.tensor.matmul(
                                o1[:, w0:512],
                                vA[:, kb, h1 * 65:(h1 + 1) * 65],
                                p4[:, 512 + w0:1024],
                                start=(kb == 0), stop=(kb == nkb - 1),
                                skip_group_check=(w0 > 0))

                        pend = None
                        for kb in range(nkb):
                            mi = 4 * qc - kb + 3
                            s2 = psS.tile([128, 1024], F32, tag="s")
                            near = mi <= 4
                            # diagonal-straddling blocks (mi<=3) only touch
                            # queries q >= w0; skip the fully-masked columns
                            w0 = 128 * (3 - mi) if mi <= 3 else 0
                            if near:
                                for hh in range(2):
                                    nc.tensor.matmul(
                                        s2[:, hh * 512 + w0:hh * 512 + 512],
                                        identf8,
                                        b_sb[:, hp, mi, hh, w0:512],
                                        start=True, stop=False)
                            for hh in range(2):
                                nc.tensor.matmul(
                                    s2[:, hh * 512 + w0:hh * 512 + 512],
                                    kT[hh * 64:hh * 64 + 64, hp,
                                       kb * 128:(kb + 1) * 128],
                                    qT[hh * 64:hh * 64 + 64, hp,
                                       qc * 512 + w0:(qc + 1) * 512],
                                    start=not near, stop=True)
                            p4 = p2p.tile([128, 1024], BF16, tag="p")
                            s2v = s2.rearrange("p (h n) -> p h n", n=512)
                            p4v = p4.rearrange("p (h n) -> p h n", n=512)
                            nc.scalar.activation(p4v[:, :, w0:512],
                                                 s2v[:, :, w0:512],
                                                 AF.Exp, scale=0.125)
                            if pend is not None:
                                issue_pv(*pend)
                            pend = (kb, p4, w0)
                        issue_pv(*pend)
                        # epilogue: normalize + drain O^T
                        for hh, ops_o in ((0, o0), (1, o1)):
                            rec = p2rc.tile([65, 512], F32, tag="rec")
                            nc.vector.reciprocal(rec[64:65, :], ops_o[64:65, :])
                            nc.sync.dma_start(out=rec_d[hp, qc, hh],
                                              in_=rec[64:65, :])
                            rep = p2rp.tile([64, 512], F32, tag="rep")
                            src = rec_d[hp, qc, hh, :]
                            nc.sync.dma_start(
                                out=rep,
                                in_=bass.AP(
                                    tensor=src.tensor, offset=src.offset,
                                    ap=[[0, 64]] + src.ap,
                                ))
                            nc.vector.tensor_tensor(
                                out=oT[hh * 64:(hh + 1) * 64, hp,
                                       qc * 512:(qc + 1) * 512],
                                in0=ops_o[0:64, :], in1=rep,
                                op=mybir.AluOpType.mult)

                # ---------------- stage 3: output projection
                with (
                    tc.tile_pool(name="s3o", bufs=3) as p3o,
                    tc.tile_pool(name="psF", bufs=4, space="PSUM") as psF,
                ):
                    for t in range(NT):
                        oev = p3o.tile([128, E], F32, tag="oev")
                        for ec in range(2):
                            ops = psF.tile([128, 512], F32, tag="ops")
                            for hp in range(4):
                                nc.tensor.matmul(
                                    ops, oT[:, hp, t * 128:(t + 1) * 128],
                                    wo_sb[:, hp, ec * 512:(ec + 1) * 512],
                                    start=(hp == 0), stop=(hp == 3))
                            nc.scalar.copy(
                                oev[:, ec * 512:(ec + 1) * 512], ops)
                        nc.sync.dma_start(
                            out=out_d[t * 128:(t + 1) * 128, :], in_=oev)

    nc.compile()
    return nc


def _get_nc():
    if "nc" not in _NC_CACHE:
        _NC_CACHE["nc"] = _build_nc()
    return _NC_CACHE["nc"]


def kernel(inputs_q, inputs_kv, mask, Wq, Wk, Wv, Wo, rel_bias):
    nc = _get_nc()
    in_maps = make_in_maps(inputs_q, inputs_kv, Wq, Wk, Wv, Wo, rel_bias)
    res = run_bass_kernel_spmd(nc, in_maps, core_ids=list(range(8)))
    out = np.stack(
        [res.results[2 * b]["out"] + res.results[2 * b + 1]["out"]
         for b in range(B)])
    return out.astype(np.float32)


# revision 9
# speedup vs baseline: 1.5586x; 1.0844x over previous
"""T5-style causal multi-head attention (B=4, S=2048, E=1024, H=16, D=64)
on 8 NeuronCores. Sharding: core c handles batch c//2 and head half c%2
(8 heads). Host sums the two row-parallel partial output projections per
batch.

v2 design notes:
- The T5 bias saturates at bucket 31 for distance >= 113; that far-field
  value is constant per head across all keys of a query row, so it cancels
  in softmax. Far blocks therefore need NO bias at all; near blocks
  (mi <= 4) add a small fp8 table of 8*(bias[bucket]-bias[31]) (+ mask)
  via an fp8 identity-matmul PSUM preload. The near table for all 4 head
  pairs is SBUF-resident (no per-hp DMA bubbles).
- QK scores for both heads accumulate into one 2-bank PSUM tile
  [128,1024]; a single ACT instruction does exp over both heads.
- Softmax denominators come free via a ones-column in the PV lhsT (m=65);
  normalization happens per (hp,qc) tile: DVE reciprocal of the den row,
  DRAM-broadcast of the reciprocal, and a fused multiply during the PSUM
  drain of O^T.
- Stage 1: PE transposes x (f32r, 1.5 cyc/row); Q path in bf16, K/V path
  in f32r. PSUM->SBUF copies are split between ACT and DVE engines.
"""
import sys

sys.path.insert(0, "/opt/trn_rl_repo")

import numpy as np
import ml_dtypes

import concourse.bass as bass
import concourse.mybir as mybir
import concourse.tile as tile
from concourse import bacc
from concourse.bass_utils import run_bass_kernel_spmd
from concourse.masks import make_identity

F32, F32R, BF16 = mybir.dt.float32, mybir.dt.float32r, mybir.dt.bfloat16
F8 = mybir.dt.float8e4
AF = mybir.ActivationFunctionType

B, S, E, H, D = 4, 2048, 1024, 16, 64
HL = H // 2          # heads per core
HD = HL * D          # 512, per-core head dims
NUM_BUCKETS, MAX_DISTANCE = 32, 128
NEG8 = np.float32(-240.0)   # min-ish of fp8 e4m3 (IEEE): kills exp after /8
NT = S // 128        # 16 token blocks
NE = E // 8 // 16    # placeholder; real NE below
NE = E // 128        # 8 embed chunks
NQ = 4               # token quads (512 tokens each)

_NC_CACHE = {}


# ---------------------------------------------------------------- host side

def _np_bucket(distance):
    """Mirror reference._relative_position_bucket for causal (distance>=0),
    float32 arithmetic like jnp."""
    max_exact = NUM_BUCKETS // 2  # 16
    is_small = distance < max_exact
    safe = np.maximum(distance, 1).astype(np.float32)
    log_scale = np.log(safe / np.float32(max_exact)).astype(np.float32) / np.float32(
        np.log(np.float32(MAX_DISTANCE / max_exact))
    )
    large = max_exact + (log_scale * np.float32(NUM_BUCKETS - max_exact)).astype(
        np.int32
    )
    large = np.minimum(large, NUM_BUCKETS - 1)
    return np.where(is_small, distance, large)


def _build_btab_near(rel_bias_half):
    """rel_bias_half [8, 32] -> near-table [128 k, 4 hp, 5 mi, 2 h, 512 q]
    fp8, holding 8*(bias[bucket] - bias[31]) for valid, -240 for masked.
    The -bias[31] shift is the constant far-field bias, which cancels in
    softmax. m-index mi = (4*qc - kb) + 3; only mi <= 4 blocks need it."""
    rb = np.asarray(rel_bias_half, dtype=np.float32)        # [8, 32]
    qq = np.arange(512)[None, :]
    kk = np.arange(128)[:, None]
    tiles = []
    for mi in range(5):
        m = mi - 3
        dd = 128 * m + qq - kk                              # [128, 512]
        bucket = _np_bucket(np.maximum(dd, 0))
        vals = 8.0 * (rb[:, bucket] - rb[:, 31][:, None, None])   # [8,128,512]
        vals = np.where(dd[None] >= 0, vals, NEG8)
        tiles.append(vals.astype(np.float32))
    t = np.stack(tiles, axis=0)                             # [5, 8h, 128, 512]
    t = t.reshape(5, 4, 2, 128, 512).transpose(3, 1, 0, 2, 4)  # [128,4,5,2,512]
    return np.ascontiguousarray(t).astype(ml_dtypes.bfloat16)


def make_in_maps(inputs_q, inputs_kv, Wq, Wk, Wv, Wo, rel_bias):
    inputs_q = np.asarray(inputs_q, dtype=np.float32)
    inputs_kv = np.asarray(inputs_kv, dtype=np.float32)
    Wq = np.asarray(Wq, dtype=np.float32)
    Wk = np.asarray(Wk, dtype=np.float32)
    Wv = np.asarray(Wv, dtype=np.float32)
    Wo = np.asarray(Wo, dtype=np.float32)
    rel_bias = np.asarray(rel_bias, dtype=np.float32)
    btabs = [_build_btab_near(rel_bias[0:HL]), _build_btab_near(rel_bias[HL:])]
    in_maps = []
    for c in range(8):
        b, half = c // 2, c % 2
        sl = slice(half * HD, (half + 1) * HD)
        in_maps.append({
            "xqT": np.ascontiguousarray(inputs_q[b].T).astype(
                ml_dtypes.bfloat16),
            "xkvT": np.ascontiguousarray(inputs_kv[b].T),
            "wq": np.ascontiguousarray(Wq[:, sl]).astype(ml_dtypes.bfloat16),
            "wk": np.ascontiguousarray(Wk[:, sl]),
            "wv": np.ascontiguousarray(Wv[:, sl]),
            "wo": np.ascontiguousarray(Wo[sl, :]),
            "btab": btabs[half],
        })
    return in_maps


# -------------------------------------------------------------- device side

def _build_nc():
    nc = bacc.Bacc(None, target_bir_lowering=False)
    xqT_d = nc.dram_tensor("xqT", [E, S], BF16, kind="ExternalInput")
    xkvT_d = nc.dram_tensor("xkvT", [E, S], F32, kind="ExternalInput")
    wq_d = nc.dram_tensor("wq", [E, HD], BF16, kind="ExternalInput")
    wk_d = nc.dram_tensor("wk", [E, HD], F32, kind="ExternalInput")
    wv_d = nc.dram_tensor("wv", [E, HD], F32, kind="ExternalInput")
    wo_d = nc.dram_tensor("wo", [HD, E], F32, kind="ExternalInput")
    btab_d = nc.dram_tensor("btab", [128, 4, 5, 2, 512], BF16,
                            kind="ExternalInput")
    out_d = nc.dram_tensor("out", [S, E], F32, kind="ExternalOutput")
    rec_d = nc.dram_tensor("rec_scratch", [4, 4, 2, 512], F32)

    with tile.TileContext(nc) as tc:
        with (
            tc.tile_pool(name="const", bufs=1) as pconst,
            tc.tile_pool(name="persist", bufs=1) as pper,
        ):
            ident = pconst.tile([128, 128], F32)
            make_identity(nc, ident)
            identb = pconst.tile([128, 128], BF16)
            nc.vector.tensor_copy(identb, ident)

            qT = pper.tile([128, 4, S], BF16)         # [pair-dims, hp, tok]
            kT = pper.tile([128, 4, S], BF16)
            vA = pper.tile([128, NT, HL * 65], BF16)  # v + ones col per head

            vAr = vA.rearrange("p t (h c) -> p t h c", c=65)
            nc.vector.memset(vAr[:, :, :, 64:65], 1.0)

            # ---------------- stage 1: projections from host-transposed x
            with (
                tc.tile_pool(name="s1w", bufs=1) as p1w,
                tc.tile_pool(name="s1xq", bufs=2) as p1xq,
                tc.tile_pool(name="s1xv", bufs=2) as p1xv,
                tc.tile_pool(name="psP", bufs=4, space="PSUM") as psP,
            ):
                wq_sb = p1w.tile([128, NE, HD], BF16)
                wk_sb = p1w.tile([128, NE, HD], F32R)
                wv_sb = p1w.tile([128, NE, HD], F32R)
                nc.sync.dma_start(
                    out=wq_sb, in_=wq_d[:].rearrange("(e p) n -> p e n", p=128))
                nc.sync.dma_start(
                    out=wk_sb, in_=wk_d[:].bitcast(F32R).rearrange(
                        "(e p) n -> p e n", p=128))
                nc.sync.dma_start(
                    out=wv_sb, in_=wv_d[:].bitcast(F32R).rearrange(
                        "(e p) n -> p e n", p=128))
                xqT_r = xqT_d[:].rearrange("(e p) s -> p e s", p=128)
                xkvT_r = xkvT_d[:].bitcast(F32R).rearrange(
                    "(e p) s -> p e s", p=128)

                # pass A: q projection (bf16)
                for tq in range(NQ):
                    sl = slice(tq * 512, (tq + 1) * 512)
                    xTq = p1xq.tile([128, NE, 512], BF16, tag="xq")
                    nc.sync.dma_start(out=xTq, in_=xqT_r[:, :, sl])
                    for hc in range(4):
                        qps = psP.tile([128, 512], F32, tag="pj")
                        for e in range(NE):
                            nc.tensor.matmul(
                                qps,
                                wq_sb[:, e, hc * 128:(hc + 1) * 128],
                                xTq[:, e, :],
                                start=(e == 0), stop=(e == NE - 1))
                        nc.vector.tensor_copy(
                            qT[:, hc, sl], qps)

                # pass B: k and v projections (f32r, shared xT tile)
                for tq in range(NQ):
                    sl = slice(tq * 512, (tq + 1) * 512)
                    xTv = p1xv.tile([128, NE, 512], F32R, tag="xv")
                    nc.sync.dma_start(out=xTv, in_=xkvT_r[:, :, sl])
                    for hc in range(4):
                        kps = psP.tile([128, 512], F32, tag="pj")
                        for e in range(NE):
                            nc.tensor.matmul(
                                kps,
                                wk_sb[:, e, hc * 128:(hc + 1) * 128],
                                xTv[:, e, :],
                                start=(e == 0), stop=(e == NE - 1))
                        nc.vector.tensor_copy(
                            kT[:, hc, sl], kps)
                    for j in range(4):
                        t = tq * 4 + j
                        vps = psP.tile([128, HD], F32, tag="pj")
                        for e in range(NE):
                            nc.tensor.matmul(
                                vps, xTv[:, e, j * 128:(j + 1) * 128],
                                wv_sb[:, e, :],
                                start=(e == 0), stop=(e == NE - 1))
                        nc.vector.tensor_copy(
                            vAr[:, t, :, 0:64],
                            vps.rearrange("p (h c) -> p h c", c=64))

            # ---------------- stages 2+3
            with tc.tile_pool(name="s2per", bufs=1) as p2per:
                oT = p2per.tile([128, 4, S], F32R)
                b_sb = p2per.tile([128, 4, 5, 2, 512], BF16)
                nc.sync.dma_start(out=b_sb, in_=btab_d[:])
                wo_sb = p2per.tile([128, 4, E], F32R)
                nc.sync.dma_start(
                    out=wo_sb,
                    in_=wo_d[:].bitcast(F32R).rearrange(
                        "(g p) n -> p g n", p=128))

                with (
                    tc.tile_pool(name="s2p", bufs=3) as p2p,
                    tc.tile_pool(name="s2rec", bufs=2) as p2rc,
                    tc.tile_pool(name="s2rep", bufs=2) as p2rp,
                    tc.tile_pool(name="psS", bufs=2, space="PSUM") as psS,
                    tc.tile_pool(name="psO", bufs=4, space="PSUM") as psO,
                ):
                 for hp in range(4):
                    for qc in range(4):
                        o0 = psO.tile([65, 512], F32, tag="o")
                        o1 = psO.tile([65, 512], F32, tag="o")
                        nkb = 4 * qc + 4
                        h0, h1 = 2 * hp, 2 * hp + 1

                        def issue_pv(kb, p4, w0, o0=o0, o1=o1, nkb=nkb,
                                     h0=h0, h1=h1):
                            nc.tensor.matmul(
                                o0[:, w0:512],
                                vA[:, kb, h0 * 65:(h0 + 1) * 65],
                                p4[:, w0:512],
                                start=(kb == 0), stop=(kb == nkb - 1),
                                skip_group_check=(w0 > 0))
                            nc.tensor.matmul(
                                o1[:, w0:512],
                                vA[:, kb, h1 * 65:(h1 + 1) * 65],
                                p4[:, 512 + w0:1024],
                                start=(kb == 0), stop=(kb == nkb - 1),
                                skip_group_check=(w0 > 0))

                        pend = None
                        for kb in range(nkb):
                            mi = 4 * qc - kb + 3
                            s2 = psS.tile([128, 1024], F32, tag="s")
                            near = mi <= 4
                            # diagonal-straddling blocks (mi<=3) only touch
                            # queries q >= w0; skip the fully-masked columns
                            w0 = 128 * (3 - mi) if mi <= 3 else 0
                            if near:
                                for hh in range(2):
                                    nc.tensor.matmul(
                                        s2[:, hh * 512 + w0:hh * 512 + 512],
                                        identb,
                                        b_sb[:, hp, mi, hh, w0:512],
                                        start=True, stop=False)
                            for hh in range(2):
                                nc.tensor.matmul(
                                    s2[:, hh * 512 + w0:hh * 512 + 512],
                                    kT[hh * 64:hh * 64 + 64, hp,
                                       kb * 128:(kb + 1) * 128],
                                    qT[hh * 64:hh * 64 + 64, hp,
                                       qc * 512 + w0:(qc + 1) * 512],
                                    start=not near, stop=True)
                            p4 = p2p.tile([128, 1024], BF16, tag="p")
                            s2v = s2.rearrange("p (h n) -> p h n", n=512)
                            p4v = p4.rearrange("p (h n) -> p h n", n=512)
                            nc.scalar.activation(p4v[:, :, w0:512],
                                                 s2v[:, :, w0:512],
                                                 AF.Exp, scale=0.125)
                            if pend is not None:
                                issue_pv(*pend)
                            pend = (kb, p4, w0)
                        issue_pv(*pend)
                        # epilogue: normalize + drain O^T
                        for hh, ops_o in ((0, o0), (1, o1)):
                            rec = p2rc.tile([65, 512], F32, tag="rec")
                            nc.vector.reciprocal(rec[64:65, :], ops_o[64:65, :])
                            nc.sync.dma_start(out=rec_d[hp, qc, hh],
                                              in_=rec[64:65, :])
                            rep = p2rp.tile([64, 512], F32, tag="rep")
                            src = rec_d[hp, qc, hh, :]
                            nc.sync.dma_start(
                                out=rep,
                                in_=bass.AP(
                                    tensor=src.tensor, offset=src.offset,
                                    ap=[[0, 64]] + src.ap,
                                ))
                            nc.vector.tensor_tensor(
                                out=oT[hh * 64:(hh + 1) * 64, hp,
                                       qc * 512:(qc + 1) * 512],
                                in0=ops_o[0:64, :], in1=rep,
                                op=mybir.AluOpType.mult)

                # ---------------- stage 3: output projection
                with (
                    tc.tile_pool(name="s3o", bufs=3) as p3o,
                    tc.tile_pool(name="psF", bufs=4, space="PSUM") as psF,
                ):
                    for t in range(NT):
                        oev = p3o.tile([128, E], F32, tag="oev")
                        for ec in range(2):
                            ops = psF.tile([128, 512], F32, tag="ops")
                            for hp in range(4):
                                nc.tensor.matmul(
                                    ops, oT[:, hp, t * 128:(t + 1) * 128],
                                    wo_sb[:, hp, ec * 512:(ec + 1) * 512],
                                    start=(hp == 0), stop=(hp == 3))
                            nc.scalar.copy(
                                oev[:, ec * 512:(ec + 1) * 512], ops)
                        nc.sync.dma_start(
                            out=out_d[t * 128:(t + 1) * 128, :], in_=oev)

    nc.compile()
    return nc


def _get_nc():
    if "nc" not in _NC_CACHE:
        _NC_CACHE["nc"] = _build_nc()
    return _NC_CACHE["nc"]


def kernel(inputs_q, inputs_kv, mask, Wq, Wk, Wv, Wo, rel_bias):
    nc = _get_nc()
    in_maps = make_in_maps(inputs_q, inputs_kv, Wq, Wk, Wv, Wo, rel_bias)
    res = run_bass_kernel_spmd(nc, in_maps, core_ids=list(range(8)))
    out = np.stack(
        [res.results[2 * b]["out"] + res.results[2 * b + 1]["out"]
         for b in range(B)])
    return out.astype(np.float32)
